# revision 3
# baseline (speedup 1.0000x reference)
"""BitLinear (ternary-weight linear) Trainium2 kernel.

Math (matching the reference):
    s      = max(act_scale, 1e-5)
    x_int  = clip(round(x / s), -127, 127)          # int8-valued
    out    = (x_int * s) @ ((packed_w - 1) * alpha).T + bias
           = (alpha * s) * (x_int @ sign(W).T) + bias

Device strategy (8 cores, data-parallel over the 16384 tokens):
  - Each core gets 2048 tokens of x (f32), plus replicated W^T in bf16
    (values in {-1, 0, +1} -> exact in bf16) and a partition-replicated bias.
  - On-chip quantization produces x_int exactly as bf16 integers
    (|x_int| <= 127 -> exact in bf16). The matmul x_int @ signW^T is then
    exact integer arithmetic in the f32 PSUM accumulator (partial sums
    < 2^24), so the only rounding vs the reference is the final
    (alpha*s) scaling - relative error ~1e-7.
  - The hardware f32->int conversion rounds to nearest-even, exactly
    matching jnp.round, so quantization is: ACT scales by 1/s and
    converts f32->int16 (RNE), then one DVE op clamps to [-127, 127]
    and converts int16->bf16. (CoreSim truncates here instead - known
    sim-vs-HW divergence; hardware is truth.)
  - x_int (bf16, token-major) is bounced through a DRAM scratch tensor so
    the xbar DMA-transpose can reload it with the contract dim (IN) on
    partitions for the matmul, in a few large efficient transfers.
"""

import sys

sys.path.insert(0, "/opt/trn_rl_repo")

import numpy as np
import ml_dtypes

# ---- problem constants (hardcoded per harness contract) ----
B, S, IN, OUT = 4, 4096, 4096, 4096
TOKENS = B * S              # 16384
N_CORES = 8
T = TOKENS // N_CORES       # 2048 tokens per core
HALF = T // 2               # token-half processed per inner pass
KT = IN // 128              # 32 k-tiles (contraction)
N_CHUNK = 512               # output columns per PSUM tile
NT = OUT // N_CHUNK         # 8 n-chunks
MT = HALF // 128            # 8 m-tiles (tokens) per half
XCH = 1024                  # free-dim chunk for quantization staging


def _build_program(inv_s: float, alpha_s: float):
    import concourse.mybir as mybir
    import concourse.tile as tile
    from concourse import bacc

    nc = bacc.Bacc("TRN2", target_bir_lowering=False, debug=False,
                   num_devices=N_CORES)

    x_d = nc.dram_tensor("x", [T, IN], mybir.dt.float32, kind="ExternalInput")
    # wt[p, k, o] = sign(W)^T[k*128 + p, o]
    wt_d = nc.dram_tensor("wt", [128, KT, OUT], mybir.dt.bfloat16,
                          kind="ExternalInput")
    bias_d = nc.dram_tensor("bias", [128, OUT], mybir.dt.float32,
                            kind="ExternalInput")
    out_d = nc.dram_tensor("out", [T, OUT], mybir.dt.float32,
                           kind="ExternalOutput")
    xq_d = nc.dram_tensor("xq_scratch", [T, IN], mybir.dt.bfloat16)

    AF = mybir.ActivationFunctionType
    ALU = mybir.AluOpType

    with tile.TileContext(nc) as tc:
        with (
            tc.tile_pool(name="stage", bufs=3) as stage,
            tc.tile_pool(name="xqt", bufs=1) as xqt_pool,
            tc.tile_pool(name="wtp", bufs=2) as wt_pool,
            tc.tile_pool(name="outsb", bufs=4) as out_pool,
            tc.tile_pool(name="biasp", bufs=2) as bias_pool,
            tc.tile_pool(name="psum", bufs=4, space="PSUM") as psum_pool,
        ):
            # ---- Phase A: quantize all tokens, bounce to DRAM as bf16 ----
            for c in range(T // 128):
                for q in range(IN // XCH):
                    xt = stage.tile([128, XCH], mybir.dt.float32, tag="xf32")
                    nc.sync.dma_start(
                        xt[:], x_d.ap()[c * 128:(c + 1) * 128,
                                        q * XCH:(q + 1) * XCH])
                    t2 = stage.tile([128, XCH], mybir.dt.int16, tag="t2")
                    nc.scalar.activation(t2[:], xt[:], AF.Copy,
                                         bias=0.0, scale=float(inv_s))
                    qb = stage.tile([128, XCH], mybir.dt.bfloat16, tag="qb")
                    nc.vector.tensor_scalar(qb[:], t2[:], 127.0, -127.0,
                                            ALU.min, ALU.max)
                    nc.sync.dma_start(
                        xq_d.ap()[c * 128:(c + 1) * 128,
                                  q * XCH:(q + 1) * XCH], qb[:])

            for half in range(2):
                t0 = half * HALF
                # ---- Phase B: transpose-load xq -> xqT [128, KT*HALF] ----
                # free index = k*HALF + t  (t within this half)
                xqT = xqt_pool.tile([128, KT * HALF], mybir.dt.bfloat16,
                                    tag="xqT")
                for k in range(KT):
                    nc.sync.dma_start_transpose(
                        xqT[:, k * HALF:(k + 1) * HALF],
                        xq_d.ap()[t0:t0 + HALF, k * 128:(k + 1) * 128])

                # ---- Phase C: matmul + epilogue ----
                for n in range(NT):
                    wt = wt_pool.tile([128, KT * N_CHUNK], mybir.dt.bfloat16,
                                      tag="wt")
                    nc.sync.dma_start(
                        wt[:].rearrange("p (k o) -> p k o", k=KT),
                        wt_d.ap()[:, :, n * N_CHUNK:(n + 1) * N_CHUNK])
                    bias_t = bias_pool.tile([128, N_CHUNK], mybir.dt.float32,
                                            tag="bias")
                    nc.sync.dma_start(
                        bias_t[:], bias_d.ap()[:, n * N_CHUNK:(n + 1) * N_CHUNK])
                    for m in range(MT):
                        psum = psum_pool.tile([128, N_CHUNK], mybir.dt.float32)
                        for k in range(KT):
                            nc.tensor.matmul(
                                psum[:],
                                xqT[:, k * HALF + m * 128:
                                       k * HALF + (m + 1) * 128],
                                wt[:, k * N_CHUNK:(k + 1) * N_CHUNK],
                                start=(k == 0), stop=(k == KT - 1))
                        osb = out_pool.tile([128, N_CHUNK], mybir.dt.float32,
                                            tag="osb")
                        nc.scalar.activation(osb[:], psum[:], AF.Copy,
                                             bias=0.0, scale=float(alpha_s))
                        nc.vector.tensor_tensor(osb[:], osb[:], bias_t[:],
                                                ALU.add)
                        nc.sync.dma_start(
                            out_d.ap()[t0 + m * 128:t0 + (m + 1) * 128,
                                       n * N_CHUNK:(n + 1) * N_CHUNK],
                            osb[:])

    nc.compile()
    return nc


def kernel(x, packed_w, alpha, act_scale, bias, _trace=False):
    from concourse.bass_utils import run_bass_kernel_spmd

    x2d = np.asarray(x, dtype=np.float32).reshape(TOKENS, IN)
    s = max(float(np.asarray(act_scale)), 1e-5)
    inv_s = 1.0 / np.float32(s)
    alpha_s = float(np.float32(np.asarray(alpha, dtype=np.float32)) *
                    np.float32(s))

    # sign weights {-1,0,1} -> bf16 exact; layout wt[p, k, o] = W^T[k*128+p, o]
    w_sign = (np.asarray(packed_w, dtype=np.float32) - 1.0)          # [OUT, IN]
    wT = w_sign.T.astype(ml_dtypes.bfloat16)                         # [IN, OUT]
    whost = np.ascontiguousarray(
        wT.reshape(KT, 128, OUT).transpose(1, 0, 2))                 # [128,KT,OUT]
    bias_rep = np.ascontiguousarray(
        np.broadcast_to(np.asarray(bias, dtype=np.float32)[None, :],
                        (128, OUT)))                                 # [128, OUT]

    nc = _build_program(float(inv_s), alpha_s)

    in_maps = [
        {"x": np.ascontiguousarray(x2d[c * T:(c + 1) * T]),
         "wt": whost, "bias": bias_rep}
        for c in range(N_CORES)
    ]
    res = run_bass_kernel_spmd(nc, in_maps, list(range(N_CORES)),
                               trace=_trace)

    out = np.empty((TOKENS, OUT), dtype=np.float32)
    for c in range(N_CORES):
        out[c * T:(c + 1) * T] = res.results[c]["out"]
    out = out.reshape(B, S, OUT)
    if _trace:
        return out, res
    return out


# revision 11
# speedup vs baseline: 52.1519x; 52.1519x over previous
"""BitLinear (ternary-weight linear) Trainium2 kernel.

Math (matching the reference):
    s      = max(act_scale, 1e-5)
    x_int  = clip(round(x / s), -127, 127)          # int8-valued
    out    = (x_int * s) @ ((packed_w - 1) * alpha).T + bias
           = (alpha * s) * (x_int @ sign(W).T) + bias

Device strategy (8 cores, data-parallel over the 16384 tokens):
  - Each core gets 2048 tokens of x (f32), plus replicated W^T in bf16
    (values in {-1, 0, +1} -> exact in bf16) and a partition-replicated bias.
  - On-chip quantization produces x_int exactly as bf16 integers
    (|x_int| <= 127 -> exact in bf16). The matmul x_int @ signW^T is then
    exact integer arithmetic in the f32 PSUM accumulator (partial sums
    < 2^24), so the only rounding vs the reference is the final
    (alpha*s) scaling - relative error ~1e-7.
  - The hardware f32->int conversion rounds to nearest-even, exactly
    matching jnp.round, so quantization is: ACT scales by 1/s and
    converts f32->int16 (RNE), then one DVE op clamps to [-127, 127]
    and converts int16->bf16. (CoreSim truncates here instead - known
    sim-vs-HW divergence; hardware is truth.)
  - x_int (bf16, token-major) is bounced through a DRAM scratch tensor so
    the xbar DMA-transpose can reload it with the contract dim (IN) on
    partitions for the matmul, in a few large efficient transfers.
"""

import sys

sys.path.insert(0, "/opt/trn_rl_repo")

import numpy as np
import ml_dtypes

# ---- problem constants (hardcoded per harness contract) ----
B, S, IN, OUT = 4, 4096, 4096, 4096
TOKENS = B * S              # 16384
N_CORES = 8
T = TOKENS // N_CORES       # 2048 tokens per core
HALF = T // 2               # token-half processed per inner pass
KT = IN // 128              # 32 k-tiles (contraction)
N_CHUNK = 512               # output columns per PSUM tile
NT = OUT // N_CHUNK         # 8 n-chunks
MT = HALF // 128            # 8 m-tiles (tokens) per half
XCH = 2048                  # free-dim chunk for quantization staging


def _build_program(inv_s: float, alpha_s: float, reps: int = 1):
    import concourse.mybir as mybir
    import concourse.tile as tile
    from concourse import bacc

    nc = bacc.Bacc("TRN2", target_bir_lowering=False, debug=False,
                   num_devices=N_CORES)

    x_d = nc.dram_tensor("x", [T, IN], mybir.dt.float32, kind="ExternalInput")
    # wt[p, k, o] = sign(W)^T[k*128 + p, o]
    wt_d = nc.dram_tensor("wt", [128, KT, OUT], mybir.dt.bfloat16,
                          kind="ExternalInput")
    bias_d = nc.dram_tensor("bias", [128, OUT], mybir.dt.float32,
                            kind="ExternalInput")
    out_d = nc.dram_tensor("out", [T, OUT], mybir.dt.float32,
                           kind="ExternalOutput")
    xq_d = nc.dram_tensor("xq_scratch", [T, IN], mybir.dt.bfloat16)

    AF = mybir.ActivationFunctionType
    ALU = mybir.AluOpType

    with tile.TileContext(nc) as tc:
        with (
            tc.tile_pool(name="stage", bufs=2) as stage,
            tc.tile_pool(name="xqt", bufs=1) as xqt_pool,
            tc.tile_pool(name="wtp", bufs=2) as wt_pool,
            tc.tile_pool(name="outsb", bufs=4) as out_pool,
            tc.tile_pool(name="biasp", bufs=1) as bias_pool,
            tc.tile_pool(name="psum", bufs=8, space="PSUM") as psum_pool,
        ):
            bias_t = bias_pool.tile([128, OUT], mybir.dt.float32, tag="bias")
            nc.sync.dma_start(bias_t[:], bias_d.ap())

            def emit_quant(half, c):
                """Quantize one 128-token row-chunk: x -> round/clip -> bf16,
                bounce to DRAM."""
                r0 = half * HALF + c * 128
                for q in range(IN // XCH):
                    i0 = q * XCH
                    xt = stage.tile([128, XCH], mybir.dt.float32, tag="xf32")
                    nc.sync.dma_start(xt[:],
                                      x_d.ap()[r0:r0 + 128, i0:i0 + XCH])
                    t2 = stage.tile([128, XCH], mybir.dt.int16, tag="t2")
                    nc.scalar.activation(t2[:], xt[:], AF.Copy,
                                         bias=0.0, scale=float(inv_s))
                    qb = stage.tile([128, XCH], mybir.dt.bfloat16, tag="qb")
                    nc.vector.tensor_scalar(qb[:], t2[:], 127.0, -127.0,
                                            ALU.min, ALU.max)
                    nc.scalar.dma_start(xq_d.ap()[r0:r0 + 128, i0:i0 + XCH],
                                        qb[:])

            def emit_transpose(half, xqT):
                t0 = half * HALF
                for k in range(KT):
                    nc.sync.dma_start_transpose(
                        xqT[:, k * HALF:(k + 1) * HALF],
                        xq_d.ap()[t0:t0 + HALF, k * 128:(k + 1) * 128])

            def load_wt(n):
                wt = wt_pool.tile([128, KT * N_CHUNK], mybir.dt.bfloat16,
                                  tag="wt")
                nc.sync.dma_start(
                    wt[:].rearrange("p (k o) -> p k o", k=KT),
                    wt_d.ap()[:, :, n * N_CHUNK:(n + 1) * N_CHUNK])
                return wt

            def emit_nchunk(half, n, xqT, wt=None):
                t0 = half * HALF
                if wt is None:
                    wt = load_wt(n)
                for m in range(MT):
                    psum = psum_pool.tile([128, N_CHUNK], mybir.dt.float32)
                    for k in range(KT):
                        nc.tensor.matmul(
                            psum[:],
                            xqT[:, k * HALF + m * 128:
                                   k * HALF + (m + 1) * 128],
                            wt[:, k * N_CHUNK:(k + 1) * N_CHUNK],
                            start=(k == 0), stop=(k == KT - 1))
                    osb = out_pool.tile([128, N_CHUNK], mybir.dt.float32,
                                        tag="osb")
                    nc.scalar.activation(osb[:], psum[:], AF.Copy,
                                         bias=0.0, scale=float(alpha_s))
                    nc.vector.tensor_tensor(
                        osb[:], osb[:],
                        bias_t[:, n * N_CHUNK:(n + 1) * N_CHUNK], ALU.add)
                    nc.scalar.dma_start(
                        out_d.ap()[t0 + m * 128:t0 + (m + 1) * 128,
                                   n * N_CHUNK:(n + 1) * N_CHUNK],
                        osb[:])

            CH = HALF // 128            # row-chunks per half
            for _rep in range(reps):
                # prefetch the first weight chunk while phase A runs
                wt0 = load_wt(0)
                # lead-in: quantize + transpose half 0
                for c in range(CH):
                    emit_quant(0, c)
                xqT0 = xqt_pool.tile([128, KT * HALF], mybir.dt.bfloat16,
                                     tag="xqT")
                emit_transpose(0, xqT0)
                # C(half 0) with A(half 1) interleaved (one quant chunk per
                # n-chunk, so half 1's xq is ready when C(half 0) drains)
                for n in range(NT):
                    emit_nchunk(0, n, xqT0, wt=wt0 if n == 0 else None)
                    if n < CH:
                        emit_quant(1, n)
                xqT1 = xqt_pool.tile([128, KT * HALF], mybir.dt.bfloat16,
                                     tag="xqT")
                emit_transpose(1, xqT1)
                for n in range(NT):
                    emit_nchunk(1, n, xqT1)

    nc.compile()
    return nc


def kernel(x, packed_w, alpha, act_scale, bias, _trace=False):
    from concourse.bass_utils import run_bass_kernel_spmd

    x2d = np.asarray(x, dtype=np.float32).reshape(TOKENS, IN)
    s = max(float(np.asarray(act_scale)), 1e-5)
    inv_s = 1.0 / np.float32(s)
    alpha_s = float(np.float32(np.asarray(alpha, dtype=np.float32)) *
                    np.float32(s))

    # sign weights {-1,0,1} -> bf16 exact; layout wt[p, k, o] = W^T[k*128+p, o]
    w_sign = (np.asarray(packed_w, dtype=np.float32) - 1.0)          # [OUT, IN]
    wT = w_sign.T.astype(ml_dtypes.bfloat16)                         # [IN, OUT]
    whost = np.ascontiguousarray(
        wT.reshape(KT, 128, OUT).transpose(1, 0, 2))                 # [128,KT,OUT]
    bias_rep = np.ascontiguousarray(
        np.broadcast_to(np.asarray(bias, dtype=np.float32)[None, :],
                        (128, OUT)))                                 # [128, OUT]

    nc = _build_program(float(inv_s), alpha_s)

    in_maps = [
        {"x": np.ascontiguousarray(x2d[c * T:(c + 1) * T]),
         "wt": whost, "bias": bias_rep}
        for c in range(N_CORES)
    ]
    res = run_bass_kernel_spmd(nc, in_maps, list(range(N_CORES)),
                               trace=_trace)

    out = np.empty((TOKENS, OUT), dtype=np.float32)
    for c in range(N_CORES):
        out[c * T:(c + 1) * T] = res.results[c]["out"]
    out = out.reshape(B, S, OUT)
    if _trace:
        return out, res
    return out
